# revision 14
# baseline (speedup 1.0000x reference)
"""Trainium2 Bass kernel for nn_BucketedGoWatti (sparse windowed attention).

Same restructured algorithm as before (19 overlapping windows = runs of 12
consecutive 128-row chunks; per-chunk column sums give per-window softmax
stats; one output GEMM), with three structural speedups:

  1. Host-prepped layouts: H arrives pre-cast and pre-transposed (fp8 main +
     fp8 residual, natural and transposed), so the device does no DRAM->DRAM
     cast round-trip and no transposed DMA.
  2. fp8 DoubleRow matmuls with residual compensation for the two big GEMMs
     (A = W^T H^T and z = pp^T H).  Each operand is split main+residual in
     fp8; three of the four cross terms are computed (r*r dropped), which
     keeps bf16-class accuracy at 2x the bf16 MAC rate.  The softmax-weight
     chain itself (S, X, pp before quantization) stays bf16/f32 since weight
     noise propagates full-strength to the output.
  3. The per-chunk dd column sums run as fp8 DoubleRow over chunk pairs.

Sharding: 8 cores = 4 batches x 2 sequence halves (unchanged).

Scales: wk8+wkr ~ 8*Wk_core (copy 1/8), w28+w2r ~ 4*W2 (copy 1/4),
qct = q^T/16, xh = X*HV/256 (lw = 8*dd/ss), BCG = 64*Gamma (host divides z
by 64).
"""
import os
import sys

for _p in ("/opt/trn_rl_repo", "/root/.axon_site/_ro/trn_rl_repo"):
    if os.path.isdir(_p) and _p not in sys.path:
        sys.path.insert(0, _p)

import numpy as np
import ml_dtypes

import concourse.bass as bass
import concourse.mybir as mybir
import concourse.tile as tile
from concourse import bacc
from concourse.bass_utils import run_bass_kernel_spmd

F32 = mybir.dt.float32
BF16 = mybir.dt.bfloat16
FP8 = mybir.dt.float8e4
AF = mybir.ActivationFunctionType
ALU = mybir.AluOpType
DR = mybir.MatmulPerfMode.DoubleRow
E4 = ml_dtypes.float8_e4m3

B, L, D, T, DG, DP = 4, 8192, 1024, 512, 256, 256
WIN, STRIDE = 1536, 384
L_LOC, NCH, NCHP, NPAIR, NWIN = 4736, 37, 38, 19, 16
BLKS = [512] * 9 + [128]                  # 4736 j-columns per core
BCG_SLABS = [6, 10, 12, 10]               # Gamma broadcast slab rows (sum 38)


def _window_starts_eff():
    starts, s = [], 0
    while s < L:
        e = min(s + WIN, L)
        starts.append(min(s, L - WIN))   # jax dynamic_slice clamps
        if e == L:
            break
        s += STRIDE
    return starts


def _core_plan():
    starts = _window_starts_eff()
    assert len(starts) == 19
    halves = [dict(lo=0, wins=starts[0:9]), dict(lo=3456, wins=starts[9:19])]
    for h in halves:
        h["win_local"] = [(s - h["lo"]) // 128 for s in h["wins"]]
    return halves


def _build_bass(reps=1):
    nc = bacc.Bacc("TRN2", target_bir_lowering=False, debug=False)
    ht8 = nc.dram_tensor("ht8", [D, L_LOC], FP8, kind="ExternalInput")
    htr = nc.dram_tensor("htr", [D, L_LOC], FP8, kind="ExternalInput")
    hn8 = nc.dram_tensor("hn8", [L_LOC, D], FP8, kind="ExternalInput")
    hnr = nc.dram_tensor("hnr", [L_LOC, D], FP8, kind="ExternalInput")
    qct = nc.dram_tensor("qct", [DP, T], BF16, kind="ExternalInput")
    gt = nc.dram_tensor("gt", [DG, T], BF16, kind="ExternalInput")
    wk8 = nc.dram_tensor("wk8", [D, DP], FP8, kind="ExternalInput")
    wkr = nc.dram_tensor("wkr", [D, DP], FP8, kind="ExternalInput")
    w28 = nc.dram_tensor("w28", [D, DG], FP8, kind="ExternalInput")
    w2r = nc.dram_tensor("w2r", [D, DG], FP8, kind="ExternalInput")
    winT = nc.dram_tensor("winT", [NWIN, NCHP], BF16, kind="ExternalInput")
    winrow = nc.dram_tensor("winrow", [128, NCH * NWIN], BF16,
                            kind="ExternalInput")
    winrow8 = nc.dram_tensor("winrow8", [128, NPAIR * 2 * 64], FP8,
                             kind="ExternalInput")
    z_out = nc.dram_tensor("z_out", [T, D], F32, kind="ExternalOutput")
    s_out = nc.dram_tensor("s_out", [NWIN, T], F32, kind="ExternalOutput")

    with tile.TileContext(nc) as tc:
        with (
            tc.tile_pool(name="dram", bufs=1, space="DRAM") as dpool,
            tc.tile_pool(name="const", bufs=1) as cpool,
            tc.tile_pool(name="res", bufs=1) as rpool,
        ):
            # ---- constants; sync queue is reserved for the ht stream,
            # small consts ride the scalar queue, memsets go first on gpsimd
            warm_sb = cpool.tile([128, 512], BF16)
            nc.gpsimd.memset(warm_sb[:], 1.0)
            scale8_sb = cpool.tile([128, 1], F32)
            nc.gpsimd.memset(scale8_sb[:], 0.125)
            gt_sb = cpool.tile([128, 2, T], BF16)
            nc.scalar.dma_start(gt_sb[:], gt[:].rearrange("(c p) t -> p c t", p=128))
            wk8_sb = cpool.tile([128, 8, DP], FP8)
            nc.scalar.dma_start(wk8_sb[:], wk8[:].rearrange("(c p) m -> p c m", p=128))
            wkr_sb = cpool.tile([128, 8, DP], FP8)
            nc.scalar.dma_start(wkr_sb[:], wkr[:].rearrange("(c p) m -> p c m", p=128))
            w28_sb = cpool.tile([128, 8, DG], FP8)
            nc.scalar.dma_start(w28_sb[:], w28[:].rearrange("(c p) m -> p c m", p=128))
            w2r_sb = cpool.tile([128, 8, DG], FP8)
            nc.scalar.dma_start(w2r_sb[:], w2r[:].rearrange("(c p) m -> p c m", p=128))
            winrow_sb = cpool.tile([128, NCH * NWIN], BF16)
            nc.scalar.dma_start(winrow_sb[:], winrow[:])
            winrow8_sb = cpool.tile([128, NPAIR, 2, 64], FP8)
            nc.scalar.dma_start(winrow8_sb[:], winrow8[:].rearrange(
                "p (a b c) -> p a b c", b=2, c=64))
            winT_sb = cpool.tile([NWIN, NCHP], BF16)
            nc.scalar.dma_start(winT_sb[:], winT[:])
            qct_sb = cpool.tile([128, 2, T], BF16)
            nc.scalar.dma_start(qct_sb[:], qct[:].rearrange("(c p) t -> p c t", p=128))

            # ---- PE warmup on the locally-memset tile (no DMA dependency)
            with tc.tile_pool(name="warm", bufs=1, space="PSUM") as wps:
                wtile = wps.tile([128, 512], F32)
                for wi in range(12):
                    nc.tensor.matmul(wtile[:], warm_sb[:, 0:128],
                                     warm_sb[:], start=True, stop=True,
                                     skip_group_check=True)

            # ---- residents
            X_sb = rpool.tile([128, NCHP, T], BF16)     # [j%128, chunk, t]
            nc.gpsimd.memset(X_sb[:, NCH, :], 0.0)      # pad chunk 37
            hn8_sb = rpool.tile([128, NCHP, D], FP8)
            nc.gpsimd.memset(hn8_sb[:, NCH, :], 0.0)
            hnr_sb = rpool.tile([128, NCHP, D], FP8)
            nc.gpsimd.memset(hnr_sb[:, NCH, :], 0.0)

            for _rep in range(reps):
                psAcc_cm = tc.tile_pool(name="psAcc", bufs=1, space="PSUM")
                psAcc = psAcc_cm.__enter__()
                ss_acc = psAcc.tile([NWIN, T], F32, tag="ssacc")
                dd_acc = psAcc.tile([64, T], F32, tag="ddacc")
                with (
                    tc.tile_pool(name="a12", bufs=1) as apool,
                    tc.tile_pool(name="ht", bufs=4) as htpool,
                    tc.tile_pool(name="psA", bufs=2, space="PSUM") as psA,
                    tc.tile_pool(name="psS", bufs=2, space="PSUM") as psS,
                    tc.tile_pool(name="psHV", bufs=2, space="PSUM") as psHV,
                    tc.tile_pool(name="xh", bufs=3) as xhpool,
                ):
                    A1_sb = apool.tile([128, 2, L_LOC], BF16, tag="A1")
                    A2_sb = apool.tile([128, 2, L_LOC], BF16, tag="A2")
                    j0 = 0
                    xh_t = None
                    for blk, jbw in enumerate(BLKS):
                        ht8_t = htpool.tile([128, 8, 512], FP8, tag="ht8")
                        nc.sync.dma_start(
                            ht8_t[:, :, :jbw],
                            ht8[:, j0:j0 + jbw].rearrange("(c p) j -> p c j", p=128))
                        htr_t = htpool.tile([128, 8, 512], FP8, tag="htr")
                        nc.gpsimd.dma_start(
                            htr_t[:, :, :jbw],
                            htr[:, j0:j0 + jbw].rearrange("(c p) j -> p c j", p=128))
                        # A1/A2 for this block: 3-pass fp8 DoubleRow
                        for (w8sb, wrsb, dst, act_copy) in (
                            (wk8_sb, wkr_sb, A1_sb, True),
                            (w28_sb, w2r_sb, A2_sb, False),
                        ):
                            for pc in range(2):
                                ps = psA.tile([128, 512], F32, tag="psA")
                                mm = 0
                                for s in range(4):
                                    for (wsb, htt) in ((w8sb, ht8_t),
                                                       (w8sb, htr_t),
                                                       (wrsb, ht8_t)):
                                        nc.tensor.matmul(
                                            ps[:, :jbw],
                                            wsb[:, 2 * s:2 * s + 2,
                                                pc * 128:(pc + 1) * 128],
                                            htt[:, 2 * s:2 * s + 2, :jbw],
                                            start=(mm == 0), stop=(mm == 11),
                                            perf_mode=DR, skip_group_check=True)
                                        mm += 1
                                if act_copy:
                                    nc.scalar.activation(
                                        dst[:, pc, j0:j0 + jbw], ps[:, :jbw],
                                        AF.Copy, scale=scale8_sb[:])
                                else:
                                    nc.vector.tensor_scalar_mul(
                                        dst[:, pc, j0:j0 + jbw], ps[:, :jbw], 0.25)
                        # PH1 for the chunks of this block
                        c0 = j0 // 128
                        for ci in range(jbw // 128):
                            c = c0 + ci
                            if (c & 1) == 0:
                                xh_t = xhpool.tile([128, 2, T], FP8, tag="xh")
                            ps_s = psS.tile([128, T], F32, tag="psS")
                            for pc in range(2):
                                nc.tensor.matmul(
                                    ps_s[:], A1_sb[:, pc, c * 128:(c + 1) * 128],
                                    qct_sb[:, pc, :],
                                    start=(pc == 0), stop=(pc == 1),
                                    skip_group_check=True)
                            nc.scalar.activation(X_sb[:, c, :], ps_s[:], AF.Exp)
                            ps_hv = psHV.tile([128, T], F32, tag="psHV")
                            for pc in range(2):
                                nc.tensor.matmul(
                                    ps_hv[:], A2_sb[:, pc, c * 128:(c + 1) * 128],
                                    gt_sb[:, pc, :],
                                    start=(pc == 0), stop=(pc == 1),
                                    skip_group_check=True)
                            nc.vector.scalar_tensor_tensor(
                                xh_t[:, c & 1, :], X_sb[:, c, :], 1.0 / 256.0,
                                ps_hv[:], op0=ALU.mult, op1=ALU.mult)
                            nc.tensor.matmul(
                                ss_acc[:], winrow_sb[:, c * NWIN:(c + 1) * NWIN],
                                X_sb[:, c, :],
                                start=(c == 0), stop=(c == NCH - 1),
                                skip_group_check=True)
                            if (c & 1) == 1 or c == NCH - 1:
                                pr = c // 2
                                if c == NCH - 1:
                                    nc.gpsimd.memset(xh_t[:, 1, :], 0.0)
                                nc.tensor.matmul(
                                    dd_acc[:], winrow8_sb[:, pr], xh_t[:],
                                    start=(pr == 0), stop=(pr == NPAIR - 1),
                                    perf_mode=DR, skip_group_check=True)
                        j0 += jbw
                    # natural-layout H for PH3 loads after the ht stream,
                    # so it does not starve the A-phase of DMA bandwidth
                    nc.sync.dma_start(
                        hn8_sb[:, 0:NCH, :],
                        hn8[:].rearrange("(c p) d -> p c d", p=128))
                    nc.sync.dma_start(
                        hnr_sb[:, 0:NCH, :],
                        hnr[:].rearrange("(c p) d -> p c d", p=128))

                # ---- PH2: window scalars + Gamma broadcast
                with tc.tile_pool(name="bcg", bufs=4) as bcgpool:
                    bcg_tiles = []
                    with (
                        tc.tile_pool(name="sc", bufs=1) as scp,
                        tc.tile_pool(name="gamc", bufs=4) as gamcpool,
                        tc.tile_pool(name="psW", bufs=2, space="PSUM") as psW,
                    ):
                        rec_sb = scp.tile([NWIN, T], F32)
                        nc.vector.reciprocal(rec_sb[:], ss_acc[:])
                        lw_sb = scp.tile([NWIN, T], F32)
                        nc.vector.scalar_tensor_tensor(
                            lw_sb[:], dd_acc[:NWIN, :], 8.0, rec_sb[:],
                            op0=ALU.mult, op1=ALU.mult)
                        elw_sb = scp.tile([NWIN, T], F32)
                        nc.scalar.activation(elw_sb[:], lw_sb[:], AF.Exp)
                        gam16 = scp.tile([NWIN, T], BF16)
                        nc.vector.tensor_mul(gam16[:], elw_sb[:], rec_sb[:])
                        gdram = dpool.tile([NCHP, T], BF16)
                        # slab-wise Gamma: small first slab so PH3 starts early
                        q0 = 0
                        for qn in BCG_SLABS:
                            ps_g = psW.tile([12, T], F32, tag="psg")
                            nc.tensor.matmul(
                                ps_g[:qn, :], winT_sb[:, q0:q0 + qn], gam16[:],
                                skip_group_check=True)
                            gamc16 = gamcpool.tile([12, T], BF16,
                                                   tag="gamc")
                            nc.vector.tensor_scalar_mul(
                                gamc16[:qn, :], ps_g[:qn, :], 64.0)
                            nc.sync.dma_start(gdram[q0:q0 + qn, :],
                                              gamc16[:qn, :])
                            bt = bcgpool.tile([128, 12, T], BF16, tag="bcg")
                            nc.gpsimd.dma_start(
                                bt[:, :qn, :],
                                gdram[q0:q0 + qn, :][None, :, :].broadcast_to(
                                    [128, qn, T]))
                            bcg_tiles.append(bt)
                            q0 += qn
                        nc.sync.dma_start(s_out[:], elw_sb[:])
                    psAcc_cm.__exit__(None, None, None)

                    # ---- PH3: z = 64 * (X*Gamma)^T (Hfp8 + Hres), 3-pass DR
                    with (
                        tc.tile_pool(name="zf", bufs=3) as zfpool,
                        tc.tile_pool(name="pp", bufs=3) as pppool,
                        tc.tile_pool(name="pp8", bufs=3) as pp8pool,
                        tc.tile_pool(name="ppr", bufs=3) as pprpool,
                        tc.tile_pool(name="psZ", bufs=1, space="PSUM") as psZ,
                    ):
                        zps = []
                        for tt in range(4):
                            zp = psZ.tile([128, D], F32, tag=f"z{tt}")
                            zps.append(zp)
                        slab_of = []
                        for k, qn in enumerate(BCG_SLABS):
                            slab_of += [k] * qn
                        slab_base = [0, 6, 16, 28]
                        for pr in range(NPAIR):
                            c2 = 2 * pr
                            kb = slab_of[c2]
                            off = c2 - slab_base[kb]
                            pp_t = pppool.tile([128, 2, T], BF16, tag="pp")
                            nc.vector.tensor_mul(
                                pp_t[:], X_sb[:, c2:c2 + 2, :],
                                bcg_tiles[kb][:, off:off + 2, :])
                            pp8_t = pp8pool.tile([128, 2, T], FP8, tag="pp8")
                            nc.scalar.activation(pp8_t[:], pp_t[:], AF.Copy)
                            ppr_t = pprpool.tile([128, 2, T], FP8, tag="ppr")
                            nc.vector.tensor_sub(ppr_t[:], pp_t[:], pp8_t[:])
                            last = pr == NPAIR - 1
                            for tt in range(4):
                                for dn in range(2):
                                    for (st, mv) in (
                                        (pp8_t, hn8_sb), (pp8_t, hnr_sb),
                                        (ppr_t, hn8_sb),
                                    ):
                                        nc.tensor.matmul(
                                            zps[tt][:, dn * 512:(dn + 1) * 512],
                                            st[:, :, tt * 128:(tt + 1) * 128],
                                            mv[:, c2:c2 + 2,
                                               dn * 512:(dn + 1) * 512],
                                            start=(pr == 0 and st is pp8_t
                                                   and mv is hn8_sb),
                                            stop=(last and st is ppr_t),
                                            perf_mode=DR,
                                            skip_group_check=True)
                                if last:
                                    zf = zfpool.tile([128, D], F32, tag="zf")
                                    if tt % 2 == 0:
                                        nc.vector.tensor_copy(zf[:], zps[tt][:])
                                    else:
                                        nc.scalar.activation(zf[:], zps[tt][:],
                                                             AF.Copy)
                                    zq = (nc.sync, nc.scalar, nc.gpsimd,
                                          nc.sync)[tt]
                                    zq.dma_start(
                                        z_out[tt * 128:(tt + 1) * 128, :], zf[:])
    nc.compile()
    return nc


_NC_CACHE = None


def _get_nc():
    global _NC_CACHE
    if _NC_CACHE is None:
        _NC_CACHE = _build_bass()
    return _NC_CACHE


def _numpy_fallback(H, G, attn_mask, Wq_core, Wk_core, Wq_win, Wk_win):
    """Reference semantics in numpy; used only if attn_mask has zeros."""
    starts = _window_starts_eff()
    q_t = G @ Wq_win
    scale = D ** -0.5
    out = np.zeros((B, T, D), np.float32)
    for b in range(B):
        m = np.full((T, 1), -np.inf, np.float32)
        ssum = np.zeros((T, 1), np.float32)
        z = np.zeros((T, D), np.float32)
        q = (G[b] @ Wq_core) / np.float32(DP ** 0.5)
        for s0 in starts:
            Hk = H[b, s0:s0 + WIN, :]
            mk = attn_mask[b, s0:s0 + WIN]
            k = Hk @ Wk_core
            sc = q @ k.T
            sc = np.where(mk[None, :], sc, np.float32(-1e30))
            sc -= sc.max(axis=-1, keepdims=True)
            al = np.exp(sc)
            al /= al.sum(axis=-1, keepdims=True)
            Zk = al @ Hk
            k_w = Zk @ Wk_win
            lw = (q_t[b] * k_w).sum(-1, keepdims=True) * scale
            m_new = np.maximum(m, lw)
            em, ew = np.exp(m - m_new), np.exp(lw - m_new)
            ssum = ssum * em + ew
            z = z * em + ew * Zk
            m = m_new
        out[b] = z / (ssum + 1e-8)
    return out


def _fp8_split(x):
    """x (f32) -> (fp8 main, fp8 residual); main+res reconstructs x closely."""
    m = x.astype(E4)
    r = (x - m.astype(np.float32)).astype(E4)
    return m, r


def kernel(H, G, attn_mask, Wq_core, Wk_core, Wq_win, Wk_win):
    H = np.asarray(H, np.float32)
    G = np.asarray(G, np.float32)
    Wq_core = np.asarray(Wq_core, np.float32)
    Wk_core = np.asarray(Wk_core, np.float32)
    Wq_win = np.asarray(Wq_win, np.float32)
    Wk_win = np.asarray(Wk_win, np.float32)
    mask = np.asarray(attn_mask)
    if not mask.all():
        return _numpy_fallback(H, G, mask, Wq_core, Wk_core, Wq_win, Wk_win)

    halves = _core_plan()
    bf = ml_dtypes.bfloat16
    wk8_h, wkr_h = _fp8_split(8.0 * Wk_core)
    w2 = Wk_win @ Wq_win.T                                  # [D, DG]
    w28_h, w2r_h = _fp8_split(4.0 * w2)

    in_maps = []
    for b in range(B):
        q_coreT = np.ascontiguousarray((G[b] @ Wq_core).T / 16.0).astype(bf)
        GT_b = np.ascontiguousarray(G[b].T).astype(bf)
        for h in halves:
            wloc = h["win_local"]
            nwin = len(wloc)
            win = np.zeros((NCHP, NWIN), np.float32)
            for w, cw in enumerate(wloc):
                win[cw:cw + 12, w] = 1.0
            winT = np.ascontiguousarray(win.T)   # dummy cols all zero
            # dummy window columns get a harmless nonzero row so the window
            # sum E stays finite; winT zeros keep them out of Gamma, and the
            # host ignores their s_out rows.
            win[NCH - 1, nwin:] = 1.0
            winrow = np.zeros((128, NCH * NWIN), np.float32)
            for c in range(NCH):
                winrow[:, c * NWIN:(c + 1) * NWIN] = win[c]
            winrow8 = np.zeros((128, NPAIR * 2 * 64), np.float32)
            for pr in range(NPAIR):
                for i in range(2):
                    winrow8[:, pr * 128 + i * 64:pr * 128 + i * 64 + NWIN] = \
                        win[2 * pr + i]
            Hs = np.ascontiguousarray(H[b, h["lo"]:h["lo"] + L_LOC, :])
            hn8_h, hnr_h = _fp8_split(Hs)
            in_maps.append(dict(
                ht8=np.ascontiguousarray(hn8_h.T),
                htr=np.ascontiguousarray(hnr_h.T),
                hn8=hn8_h, hnr=hnr_h,
                qct=q_coreT, gt=GT_b,
                wk8=wk8_h, wkr=wkr_h, w28=w28_h, w2r=w2r_h,
                winT=winT.astype(bf),
                winrow=winrow.astype(bf), winrow8=winrow8.astype(E4)))

    global _last_in_maps
    _last_in_maps = in_maps
    nc = _get_nc()
    res = run_bass_kernel_spmd(nc, in_maps, core_ids=list(range(8)))
    out = np.zeros((B, T, D), np.float32)
    nw0 = len(halves[0]["win_local"])
    nw1 = len(halves[1]["win_local"])
    for b in range(B):
        r0, r1 = res.results[2 * b], res.results[2 * b + 1]
        denom = (r0["s_out"][:nw0].sum(axis=0) + r1["s_out"][:nw1].sum(axis=0)
                 + 1e-8)
        out[b] = (r0["z_out"] + r1["z_out"]) / 64.0 / denom[:, None]
    return out


# revision 15
# speedup vs baseline: 1.1004x; 1.1004x over previous
"""Trainium2 Bass kernel for nn_BucketedGoWatti (sparse windowed attention).

Same restructured algorithm as before (19 overlapping windows = runs of 12
consecutive 128-row chunks; per-chunk column sums give per-window softmax
stats; one output GEMM), with three structural speedups:

  1. Host-prepped layouts: H arrives pre-cast and pre-transposed (fp8 main +
     fp8 residual, natural and transposed), so the device does no DRAM->DRAM
     cast round-trip and no transposed DMA.
  2. fp8 DoubleRow matmuls with residual compensation for the two big GEMMs
     (A = W^T H^T and z = pp^T H).  Each operand is split main+residual in
     fp8; three of the four cross terms are computed (r*r dropped), which
     keeps bf16-class accuracy at 2x the bf16 MAC rate.  The softmax-weight
     chain itself (S, X, pp before quantization) stays bf16/f32 since weight
     noise propagates full-strength to the output.
  3. The per-chunk dd column sums run as fp8 DoubleRow over chunk pairs.

Sharding: 8 cores = 4 batches x 2 sequence halves (unchanged).

Scales: wk8+wkr ~ 8*Wk_core (copy 1/8), w28+w2r ~ 4*W2 (copy 1/4),
qct = q^T/16, xh = X*HV/256 (lw = 8*dd/ss), BCG = 64*Gamma (host divides z
by 64).
"""
import os
import sys

for _p in ("/opt/trn_rl_repo", "/root/.axon_site/_ro/trn_rl_repo"):
    if os.path.isdir(_p) and _p not in sys.path:
        sys.path.insert(0, _p)

import numpy as np
import ml_dtypes

import concourse.bass as bass
import concourse.mybir as mybir
import concourse.tile as tile
from concourse import bacc
from concourse.bass_utils import run_bass_kernel_spmd

F32 = mybir.dt.float32
BF16 = mybir.dt.bfloat16
FP8 = mybir.dt.float8e4
AF = mybir.ActivationFunctionType
ALU = mybir.AluOpType
DR = mybir.MatmulPerfMode.DoubleRow
E4 = ml_dtypes.float8_e4m3

B, L, D, T, DG, DP = 4, 8192, 1024, 512, 256, 256
WIN, STRIDE = 1536, 384
L_LOC, NCH, NCHP, NPAIR, NWIN = 4736, 37, 38, 19, 16
BLKS = [512] * 9 + [128]                  # 4736 j-columns per core
BCG_SLABS = [6, 10, 12, 10]               # Gamma broadcast slab rows (sum 38)


def _window_starts_eff():
    starts, s = [], 0
    while s < L:
        e = min(s + WIN, L)
        starts.append(min(s, L - WIN))   # jax dynamic_slice clamps
        if e == L:
            break
        s += STRIDE
    return starts


def _core_plan():
    starts = _window_starts_eff()
    assert len(starts) == 19
    halves = [dict(lo=0, wins=starts[0:9]), dict(lo=3456, wins=starts[9:19])]
    for h in halves:
        h["win_local"] = [(s - h["lo"]) // 128 for s in h["wins"]]
    return halves


def _build_bass(reps=1):
    nc = bacc.Bacc("TRN2", target_bir_lowering=False, debug=False)
    ht8 = nc.dram_tensor("ht8", [D, L_LOC], FP8, kind="ExternalInput")
    htr = nc.dram_tensor("htr", [D, L_LOC], FP8, kind="ExternalInput")
    hn8 = nc.dram_tensor("hn8", [L_LOC, D], FP8, kind="ExternalInput")
    hnr = nc.dram_tensor("hnr", [L_LOC, D], FP8, kind="ExternalInput")
    qct = nc.dram_tensor("qct", [DP, T], BF16, kind="ExternalInput")
    gt = nc.dram_tensor("gt", [DG, T], BF16, kind="ExternalInput")
    wk8 = nc.dram_tensor("wk8", [D, DP], FP8, kind="ExternalInput")
    wkr = nc.dram_tensor("wkr", [D, DP], FP8, kind="ExternalInput")
    w28 = nc.dram_tensor("w28", [D, DG], FP8, kind="ExternalInput")
    w2r = nc.dram_tensor("w2r", [D, DG], FP8, kind="ExternalInput")
    winT = nc.dram_tensor("winT", [NWIN, NCHP], BF16, kind="ExternalInput")
    winrow = nc.dram_tensor("winrow", [128, NCH * NWIN], BF16,
                            kind="ExternalInput")
    winrow8 = nc.dram_tensor("winrow8", [128, NPAIR * 2 * 64], FP8,
                             kind="ExternalInput")
    z_out = nc.dram_tensor("z_out", [T, D], F32, kind="ExternalOutput")
    s_out = nc.dram_tensor("s_out", [NWIN, T], F32, kind="ExternalOutput")

    with tile.TileContext(nc) as tc:
        with (
            tc.tile_pool(name="dram", bufs=1, space="DRAM") as dpool,
            tc.tile_pool(name="const", bufs=1) as cpool,
            tc.tile_pool(name="res", bufs=1) as rpool,
        ):
            # ---- constants; sync queue is reserved for the ht stream,
            # small consts ride the scalar queue, memsets go first on gpsimd
            warm_sb = cpool.tile([128, 512], BF16)
            nc.gpsimd.memset(warm_sb[:], 1.0)
            scale8_sb = cpool.tile([128, 1], F32)
            nc.gpsimd.memset(scale8_sb[:], 0.125)
            gt_sb = cpool.tile([128, 2, T], BF16)
            nc.scalar.dma_start(gt_sb[:], gt[:].rearrange("(c p) t -> p c t", p=128))
            wk8_sb = cpool.tile([128, 8, DP], FP8)
            nc.scalar.dma_start(wk8_sb[:], wk8[:].rearrange("(c p) m -> p c m", p=128))
            wkr_sb = cpool.tile([128, 8, DP], FP8)
            nc.scalar.dma_start(wkr_sb[:], wkr[:].rearrange("(c p) m -> p c m", p=128))
            w28_sb = cpool.tile([128, 8, DG], FP8)
            nc.scalar.dma_start(w28_sb[:], w28[:].rearrange("(c p) m -> p c m", p=128))
            w2r_sb = cpool.tile([128, 8, DG], FP8)
            nc.scalar.dma_start(w2r_sb[:], w2r[:].rearrange("(c p) m -> p c m", p=128))
            winrow_sb = cpool.tile([128, NCH * NWIN], BF16)
            nc.scalar.dma_start(winrow_sb[:], winrow[:])
            winrow8_sb = cpool.tile([128, NPAIR, 2, 64], FP8)
            nc.scalar.dma_start(winrow8_sb[:], winrow8[:].rearrange(
                "p (a b c) -> p a b c", b=2, c=64))
            winT_sb = cpool.tile([NWIN, NCHP], BF16)
            nc.scalar.dma_start(winT_sb[:], winT[:])
            qct_sb = cpool.tile([128, 2, T], BF16)
            nc.scalar.dma_start(qct_sb[:], qct[:].rearrange("(c p) t -> p c t", p=128))

            # ---- PE warmup on the locally-memset tile (no DMA dependency)
            with tc.tile_pool(name="warm", bufs=1, space="PSUM") as wps:
                wtile = wps.tile([128, 512], F32)
                for wi in range(12):
                    nc.tensor.matmul(wtile[:], warm_sb[:, 0:128],
                                     warm_sb[:], start=True, stop=True,
                                     skip_group_check=True)

            # ---- residents
            X_sb = rpool.tile([128, NCHP, T], BF16)     # [j%128, chunk, t]
            nc.gpsimd.memset(X_sb[:, NCH, :], 0.0)      # pad chunk 37
            hn8_sb = rpool.tile([128, NCHP, D], FP8)
            nc.gpsimd.memset(hn8_sb[:, NCH, :], 0.0)
            hnr_sb = rpool.tile([128, NCHP, D], FP8)
            nc.gpsimd.memset(hnr_sb[:, NCH, :], 0.0)

            for _rep in range(reps):
                psAcc_cm = tc.tile_pool(name="psAcc", bufs=1, space="PSUM")
                psAcc = psAcc_cm.__enter__()
                ss_acc = psAcc.tile([NWIN, T], F32, tag="ssacc")
                dd_acc = psAcc.tile([64, T], F32, tag="ddacc")
                with (
                    tc.tile_pool(name="a12", bufs=1) as apool,
                    tc.tile_pool(name="ht", bufs=4) as htpool,
                    tc.tile_pool(name="psA", bufs=2, space="PSUM") as psA,
                    tc.tile_pool(name="psS", bufs=2, space="PSUM") as psS,
                    tc.tile_pool(name="psHV", bufs=2, space="PSUM") as psHV,
                    tc.tile_pool(name="xh", bufs=3) as xhpool,
                ):
                    A1_sb = apool.tile([128, 2, L_LOC], BF16, tag="A1")
                    A2_sb = apool.tile([128, 2, L_LOC], BF16, tag="A2")
                    j0 = 0
                    xh_t = None
                    for blk, jbw in enumerate(BLKS):
                        ht8_t = htpool.tile([128, 8, 512], FP8, tag="ht8")
                        nc.sync.dma_start(
                            ht8_t[:, :, :jbw],
                            ht8[:, j0:j0 + jbw].rearrange("(c p) j -> p c j", p=128))
                        htr_t = htpool.tile([128, 8, 512], FP8, tag="htr")
                        nc.sync.dma_start(
                            htr_t[:, :, :jbw],
                            htr[:, j0:j0 + jbw].rearrange("(c p) j -> p c j", p=128))
                        # A1/A2 for this block: 3-pass fp8 DoubleRow
                        for (w8sb, wrsb, dst, act_copy) in (
                            (wk8_sb, wkr_sb, A1_sb, True),
                            (w28_sb, w2r_sb, A2_sb, False),
                        ):
                            for pc in range(2):
                                ps = psA.tile([128, 512], F32, tag="psA")
                                mm = 0
                                for s in range(4):
                                    for (wsb, htt) in ((w8sb, ht8_t),
                                                       (w8sb, htr_t),
                                                       (wrsb, ht8_t)):
                                        nc.tensor.matmul(
                                            ps[:, :jbw],
                                            wsb[:, 2 * s:2 * s + 2,
                                                pc * 128:(pc + 1) * 128],
                                            htt[:, 2 * s:2 * s + 2, :jbw],
                                            start=(mm == 0), stop=(mm == 11),
                                            perf_mode=DR, skip_group_check=True)
                                        mm += 1
                                if act_copy:
                                    nc.scalar.activation(
                                        dst[:, pc, j0:j0 + jbw], ps[:, :jbw],
                                        AF.Copy, scale=scale8_sb[:])
                                else:
                                    nc.vector.tensor_scalar_mul(
                                        dst[:, pc, j0:j0 + jbw], ps[:, :jbw], 0.25)
                        # PH1 for the chunks of this block
                        c0 = j0 // 128
                        for ci in range(jbw // 128):
                            c = c0 + ci
                            if (c & 1) == 0:
                                xh_t = xhpool.tile([128, 2, T], FP8, tag="xh")
                            ps_s = psS.tile([128, T], F32, tag="psS")
                            for pc in range(2):
                                nc.tensor.matmul(
                                    ps_s[:], A1_sb[:, pc, c * 128:(c + 1) * 128],
                                    qct_sb[:, pc, :],
                                    start=(pc == 0), stop=(pc == 1),
                                    skip_group_check=True)
                            nc.scalar.activation(X_sb[:, c, :], ps_s[:], AF.Exp)
                            ps_hv = psHV.tile([128, T], F32, tag="psHV")
                            for pc in range(2):
                                nc.tensor.matmul(
                                    ps_hv[:], A2_sb[:, pc, c * 128:(c + 1) * 128],
                                    gt_sb[:, pc, :],
                                    start=(pc == 0), stop=(pc == 1),
                                    skip_group_check=True)
                            nc.vector.scalar_tensor_tensor(
                                xh_t[:, c & 1, :], X_sb[:, c, :], 1.0 / 256.0,
                                ps_hv[:], op0=ALU.mult, op1=ALU.mult)
                            nc.tensor.matmul(
                                ss_acc[:], winrow_sb[:, c * NWIN:(c + 1) * NWIN],
                                X_sb[:, c, :],
                                start=(c == 0), stop=(c == NCH - 1),
                                skip_group_check=True)
                            if (c & 1) == 1 or c == NCH - 1:
                                pr = c // 2
                                if c == NCH - 1:
                                    nc.gpsimd.memset(xh_t[:, 1, :], 0.0)
                                nc.tensor.matmul(
                                    dd_acc[:], winrow8_sb[:, pr], xh_t[:],
                                    start=(pr == 0), stop=(pr == NPAIR - 1),
                                    perf_mode=DR, skip_group_check=True)
                        j0 += jbw
                    # natural-layout H for PH3 loads after the ht stream,
                    # so it does not starve the A-phase of DMA bandwidth
                    nc.sync.dma_start(
                        hn8_sb[:, 0:NCH, :],
                        hn8[:].rearrange("(c p) d -> p c d", p=128))
                    nc.sync.dma_start(
                        hnr_sb[:, 0:NCH, :],
                        hnr[:].rearrange("(c p) d -> p c d", p=128))

                # ---- PH2: window scalars + Gamma broadcast
                with tc.tile_pool(name="bcg", bufs=4) as bcgpool:
                    bcg_tiles = []
                    with (
                        tc.tile_pool(name="sc", bufs=1) as scp,
                        tc.tile_pool(name="gamc", bufs=4) as gamcpool,
                        tc.tile_pool(name="psW", bufs=2, space="PSUM") as psW,
                    ):
                        rec_sb = scp.tile([NWIN, T], F32)
                        nc.vector.reciprocal(rec_sb[:], ss_acc[:])
                        lw_sb = scp.tile([NWIN, T], F32)
                        nc.vector.scalar_tensor_tensor(
                            lw_sb[:], dd_acc[:NWIN, :], 8.0, rec_sb[:],
                            op0=ALU.mult, op1=ALU.mult)
                        elw_sb = scp.tile([NWIN, T], F32)
                        nc.scalar.activation(elw_sb[:], lw_sb[:], AF.Exp)
                        gam16 = scp.tile([NWIN, T], BF16)
                        nc.vector.tensor_mul(gam16[:], elw_sb[:], rec_sb[:])
                        gdram = dpool.tile([NCHP, T], BF16)
                        # slab-wise Gamma: small first slab so PH3 starts early
                        q0 = 0
                        for qn in BCG_SLABS:
                            ps_g = psW.tile([12, T], F32, tag="psg")
                            nc.tensor.matmul(
                                ps_g[:qn, :], winT_sb[:, q0:q0 + qn], gam16[:],
                                skip_group_check=True)
                            gamc16 = gamcpool.tile([12, T], BF16,
                                                   tag="gamc")
                            nc.vector.tensor_scalar_mul(
                                gamc16[:qn, :], ps_g[:qn, :], 64.0)
                            nc.sync.dma_start(gdram[q0:q0 + qn, :],
                                              gamc16[:qn, :])
                            bt = bcgpool.tile([128, 12, T], BF16, tag="bcg")
                            nc.gpsimd.dma_start(
                                bt[:, :qn, :],
                                gdram[q0:q0 + qn, :][None, :, :].broadcast_to(
                                    [128, qn, T]))
                            bcg_tiles.append(bt)
                            q0 += qn
                        nc.sync.dma_start(s_out[:], elw_sb[:])
                    psAcc_cm.__exit__(None, None, None)

                    # ---- PH3: z = 64 * (X*Gamma)^T (Hfp8 + Hres), 3-pass DR
                    with (
                        tc.tile_pool(name="zf", bufs=3) as zfpool,
                        tc.tile_pool(name="pp", bufs=3) as pppool,
                        tc.tile_pool(name="pp8", bufs=3) as pp8pool,
                        tc.tile_pool(name="ppr", bufs=3) as pprpool,
                        tc.tile_pool(name="psZ", bufs=1, space="PSUM") as psZ,
                    ):
                        zps = []
                        for tt in range(4):
                            zp = psZ.tile([128, D], F32, tag=f"z{tt}")
                            zps.append(zp)
                        slab_of = []
                        for k, qn in enumerate(BCG_SLABS):
                            slab_of += [k] * qn
                        slab_base = [0, 6, 16, 28]
                        for pr in range(NPAIR):
                            c2 = 2 * pr
                            kb = slab_of[c2]
                            off = c2 - slab_base[kb]
                            pp_t = pppool.tile([128, 2, T], BF16, tag="pp")
                            nc.vector.tensor_mul(
                                pp_t[:], X_sb[:, c2:c2 + 2, :],
                                bcg_tiles[kb][:, off:off + 2, :])
                            pp8_t = pp8pool.tile([128, 2, T], FP8, tag="pp8")
                            nc.scalar.activation(pp8_t[:], pp_t[:], AF.Copy)
                            ppr_t = pprpool.tile([128, 2, T], FP8, tag="ppr")
                            nc.vector.tensor_sub(ppr_t[:], pp_t[:], pp8_t[:])
                            last = pr == NPAIR - 1
                            for tt in range(4):
                                for dn in range(2):
                                    for (st, mv) in (
                                        (pp8_t, hn8_sb), (pp8_t, hnr_sb),
                                        (ppr_t, hn8_sb),
                                    ):
                                        nc.tensor.matmul(
                                            zps[tt][:, dn * 512:(dn + 1) * 512],
                                            st[:, :, tt * 128:(tt + 1) * 128],
                                            mv[:, c2:c2 + 2,
                                               dn * 512:(dn + 1) * 512],
                                            start=(pr == 0 and st is pp8_t
                                                   and mv is hn8_sb),
                                            stop=(last and st is ppr_t),
                                            perf_mode=DR,
                                            skip_group_check=True)
                                if last:
                                    zf = zfpool.tile([128, D], F32, tag="zf")
                                    if tt % 2 == 0:
                                        nc.vector.tensor_copy(zf[:], zps[tt][:])
                                    else:
                                        nc.scalar.activation(zf[:], zps[tt][:],
                                                             AF.Copy)
                                    zq = (nc.sync, nc.scalar, nc.gpsimd,
                                          nc.sync)[tt]
                                    zq.dma_start(
                                        z_out[tt * 128:(tt + 1) * 128, :], zf[:])
    nc.compile()
    return nc


_NC_CACHE = None


def _get_nc():
    global _NC_CACHE
    if _NC_CACHE is None:
        _NC_CACHE = _build_bass()
    return _NC_CACHE


def _numpy_fallback(H, G, attn_mask, Wq_core, Wk_core, Wq_win, Wk_win):
    """Reference semantics in numpy; used only if attn_mask has zeros."""
    starts = _window_starts_eff()
    q_t = G @ Wq_win
    scale = D ** -0.5
    out = np.zeros((B, T, D), np.float32)
    for b in range(B):
        m = np.full((T, 1), -np.inf, np.float32)
        ssum = np.zeros((T, 1), np.float32)
        z = np.zeros((T, D), np.float32)
        q = (G[b] @ Wq_core) / np.float32(DP ** 0.5)
        for s0 in starts:
            Hk = H[b, s0:s0 + WIN, :]
            mk = attn_mask[b, s0:s0 + WIN]
            k = Hk @ Wk_core
            sc = q @ k.T
            sc = np.where(mk[None, :], sc, np.float32(-1e30))
            sc -= sc.max(axis=-1, keepdims=True)
            al = np.exp(sc)
            al /= al.sum(axis=-1, keepdims=True)
            Zk = al @ Hk
            k_w = Zk @ Wk_win
            lw = (q_t[b] * k_w).sum(-1, keepdims=True) * scale
            m_new = np.maximum(m, lw)
            em, ew = np.exp(m - m_new), np.exp(lw - m_new)
            ssum = ssum * em + ew
            z = z * em + ew * Zk
            m = m_new
        out[b] = z / (ssum + 1e-8)
    return out


def _fp8_split(x):
    """x (f32) -> (fp8 main, fp8 residual); main+res reconstructs x closely."""
    m = x.astype(E4)
    r = (x - m.astype(np.float32)).astype(E4)
    return m, r


def kernel(H, G, attn_mask, Wq_core, Wk_core, Wq_win, Wk_win):
    H = np.asarray(H, np.float32)
    G = np.asarray(G, np.float32)
    Wq_core = np.asarray(Wq_core, np.float32)
    Wk_core = np.asarray(Wk_core, np.float32)
    Wq_win = np.asarray(Wq_win, np.float32)
    Wk_win = np.asarray(Wk_win, np.float32)
    mask = np.asarray(attn_mask)
    if not mask.all():
        return _numpy_fallback(H, G, mask, Wq_core, Wk_core, Wq_win, Wk_win)

    halves = _core_plan()
    bf = ml_dtypes.bfloat16
    wk8_h, wkr_h = _fp8_split(8.0 * Wk_core)
    w2 = Wk_win @ Wq_win.T                                  # [D, DG]
    w28_h, w2r_h = _fp8_split(4.0 * w2)

    in_maps = []
    for b in range(B):
        q_coreT = np.ascontiguousarray((G[b] @ Wq_core).T / 16.0).astype(bf)
        GT_b = np.ascontiguousarray(G[b].T).astype(bf)
        for h in halves:
            wloc = h["win_local"]
            nwin = len(wloc)
            win = np.zeros((NCHP, NWIN), np.float32)
            for w, cw in enumerate(wloc):
                win[cw:cw + 12, w] = 1.0
            winT = np.ascontiguousarray(win.T)   # dummy cols all zero
            # dummy window columns get a harmless nonzero row so the window
            # sum E stays finite; winT zeros keep them out of Gamma, and the
            # host ignores their s_out rows.
            win[NCH - 1, nwin:] = 1.0
            winrow = np.zeros((128, NCH * NWIN), np.float32)
            for c in range(NCH):
                winrow[:, c * NWIN:(c + 1) * NWIN] = win[c]
            winrow8 = np.zeros((128, NPAIR * 2 * 64), np.float32)
            for pr in range(NPAIR):
                for i in range(2):
                    winrow8[:, pr * 128 + i * 64:pr * 128 + i * 64 + NWIN] = \
                        win[2 * pr + i]
            Hs = np.ascontiguousarray(H[b, h["lo"]:h["lo"] + L_LOC, :])
            hn8_h, hnr_h = _fp8_split(Hs)
            in_maps.append(dict(
                ht8=np.ascontiguousarray(hn8_h.T),
                htr=np.ascontiguousarray(hnr_h.T),
                hn8=hn8_h, hnr=hnr_h,
                qct=q_coreT, gt=GT_b,
                wk8=wk8_h, wkr=wkr_h, w28=w28_h, w2r=w2r_h,
                winT=winT.astype(bf),
                winrow=winrow.astype(bf), winrow8=winrow8.astype(E4)))

    global _last_in_maps
    _last_in_maps = in_maps
    nc = _get_nc()
    res = run_bass_kernel_spmd(nc, in_maps, core_ids=list(range(8)))
    out = np.zeros((B, T, D), np.float32)
    nw0 = len(halves[0]["win_local"])
    nw1 = len(halves[1]["win_local"])
    for b in range(B):
        r0, r1 = res.results[2 * b], res.results[2 * b + 1]
        denom = (r0["s_out"][:nw0].sum(axis=0) + r1["s_out"][:nw1].sum(axis=0)
                 + 1e-8)
        out[b] = (r0["z_out"] + r1["z_out"]) / 64.0 / denom[:, None]
    return out


# revision 18
# speedup vs baseline: 2.5812x; 2.3458x over previous
"""Trainium2 Bass kernel for nn_BucketedGoWatti (sparse windowed attention).

Same restructured algorithm as before (19 overlapping windows = runs of 12
consecutive 128-row chunks; per-chunk column sums give per-window softmax
stats; one output GEMM), with three structural speedups:

  1. Host-prepped layouts: H arrives pre-cast and pre-transposed (fp8 main +
     fp8 residual, natural and transposed), so the device does no DRAM->DRAM
     cast round-trip and no transposed DMA.
  2. fp8 DoubleRow matmuls with residual compensation for the two big GEMMs
     (A = W^T H^T and z = pp^T H).  Each operand is split main+residual in
     fp8; three of the four cross terms are computed (r*r dropped), which
     keeps bf16-class accuracy at 2x the bf16 MAC rate.  The softmax-weight
     chain itself (S, X, pp before quantization) stays bf16/f32 since weight
     noise propagates full-strength to the output.
  3. The per-chunk dd column sums run as fp8 DoubleRow over chunk pairs.

Sharding: 8 cores = 4 batches x 2 sequence halves (unchanged).

Scales: wk8+wkr ~ 8*Wk_core (copy 1/8), w28+w2r ~ 4*W2 (copy 1/4),
qct = q^T/16, xh = X*HV/256 (lw = 8*dd/ss), BCG = 64*Gamma (host divides z
by 64).
"""
import os
import sys

for _p in ("/opt/trn_rl_repo", "/root/.axon_site/_ro/trn_rl_repo"):
    if os.path.isdir(_p) and _p not in sys.path:
        sys.path.insert(0, _p)

import numpy as np
import ml_dtypes

import concourse.bass as bass
import concourse.mybir as mybir
import concourse.tile as tile
from concourse import bacc
from concourse.bass_utils import run_bass_kernel_spmd

F32 = mybir.dt.float32
BF16 = mybir.dt.bfloat16
FP8 = mybir.dt.float8e4
AF = mybir.ActivationFunctionType
ALU = mybir.AluOpType
DR = mybir.MatmulPerfMode.DoubleRow
E4 = ml_dtypes.float8_e4m3

B, L, D, T, DG, DP = 4, 8192, 1024, 512, 256, 256
WIN, STRIDE = 1536, 384
L_LOC, NCH, NCHP, NPAIR, NWIN = 4736, 37, 38, 19, 16
BLKS = [512] * 9 + [128]                  # 4736 j-columns per core
BCG_SLABS = [6, 10, 12, 10]               # Gamma broadcast slab rows (sum 38)


def _window_starts_eff():
    starts, s = [], 0
    while s < L:
        e = min(s + WIN, L)
        starts.append(min(s, L - WIN))   # jax dynamic_slice clamps
        if e == L:
            break
        s += STRIDE
    return starts


def _core_plan():
    starts = _window_starts_eff()
    assert len(starts) == 19
    halves = [dict(lo=0, wins=starts[0:9]), dict(lo=3456, wins=starts[9:19])]
    for h in halves:
        h["win_local"] = [(s - h["lo"]) // 128 for s in h["wins"]]
    return halves


def _build_bass(reps=1):
    nc = bacc.Bacc("TRN2", target_bir_lowering=False, debug=False)
    ht8 = nc.dram_tensor("ht8", [D, L_LOC], FP8, kind="ExternalInput")
    htr = nc.dram_tensor("htr", [D, L_LOC], FP8, kind="ExternalInput")
    hn8 = nc.dram_tensor("hn8", [L_LOC, D], FP8, kind="ExternalInput")
    hnr = nc.dram_tensor("hnr", [L_LOC, D], FP8, kind="ExternalInput")
    qct = nc.dram_tensor("qct", [DP, T], BF16, kind="ExternalInput")
    gt = nc.dram_tensor("gt", [DG, T], BF16, kind="ExternalInput")
    wk8 = nc.dram_tensor("wk8", [D, DP], FP8, kind="ExternalInput")
    wkr = nc.dram_tensor("wkr", [D, DP], FP8, kind="ExternalInput")
    w28 = nc.dram_tensor("w28", [D, DG], FP8, kind="ExternalInput")
    w2r = nc.dram_tensor("w2r", [D, DG], FP8, kind="ExternalInput")
    winT = nc.dram_tensor("winT", [NWIN, NCHP], BF16, kind="ExternalInput")
    winrow = nc.dram_tensor("winrow", [128, NCH * NWIN], BF16,
                            kind="ExternalInput")
    winrow8 = nc.dram_tensor("winrow8", [128, NPAIR * 2 * 64], FP8,
                             kind="ExternalInput")
    z_out = nc.dram_tensor("z_out", [T, D], BF16, kind="ExternalOutput")
    s_out = nc.dram_tensor("s_out", [NWIN, T], F32, kind="ExternalOutput")

    with tile.TileContext(nc) as tc:
        with (
            tc.tile_pool(name="dram", bufs=1, space="DRAM") as dpool,
            tc.tile_pool(name="const", bufs=1) as cpool,
            tc.tile_pool(name="res", bufs=1) as rpool,
        ):
            # ---- constants; sync queue is reserved for the ht stream,
            # small consts ride the scalar queue, memsets go first on gpsimd
            warm_sb = cpool.tile([128, 512], BF16)
            nc.gpsimd.memset(warm_sb[:], 1.0)
            scale8_sb = cpool.tile([128, 1], F32)
            nc.gpsimd.memset(scale8_sb[:], 0.125)
            gt_sb = cpool.tile([128, 2, T], BF16)
            nc.scalar.dma_start(gt_sb[:], gt[:].rearrange("(c p) t -> p c t", p=128))
            wk8_sb = cpool.tile([128, 8, DP], FP8)
            nc.scalar.dma_start(wk8_sb[:], wk8[:].rearrange("(c p) m -> p c m", p=128))
            wkr_sb = cpool.tile([128, 8, DP], FP8)
            nc.scalar.dma_start(wkr_sb[:], wkr[:].rearrange("(c p) m -> p c m", p=128))
            w28_sb = cpool.tile([128, 8, DG], FP8)
            nc.scalar.dma_start(w28_sb[:], w28[:].rearrange("(c p) m -> p c m", p=128))
            w2r_sb = cpool.tile([128, 8, DG], FP8)
            nc.scalar.dma_start(w2r_sb[:], w2r[:].rearrange("(c p) m -> p c m", p=128))
            winrow_sb = cpool.tile([128, NCH * NWIN], BF16)
            nc.scalar.dma_start(winrow_sb[:], winrow[:])
            winrow8_sb = cpool.tile([128, NPAIR, 2, 64], FP8)
            nc.scalar.dma_start(winrow8_sb[:], winrow8[:].rearrange(
                "p (a b c) -> p a b c", b=2, c=64))
            winT_sb = cpool.tile([NWIN, NCHP], BF16)
            nc.scalar.dma_start(winT_sb[:], winT[:])
            qct_sb = cpool.tile([128, 2, T], BF16)
            nc.scalar.dma_start(qct_sb[:], qct[:].rearrange("(c p) t -> p c t", p=128))

            # ---- PE warmup on the locally-memset tile (no DMA dependency)
            with tc.tile_pool(name="warm", bufs=1, space="PSUM") as wps:
                wtile = wps.tile([128, 512], F32)
                for wi in range(12):
                    nc.tensor.matmul(wtile[:], warm_sb[:, 0:128],
                                     warm_sb[:], start=True, stop=True,
                                     skip_group_check=True)

            # ---- residents
            X_sb = rpool.tile([128, NCHP, T], BF16)     # [j%128, chunk, t]
            nc.gpsimd.memset(X_sb[:, NCH, :], 0.0)      # pad chunk 37
            hn8_sb = rpool.tile([128, NCHP, D], FP8)
            nc.gpsimd.memset(hn8_sb[:, NCH, :], 0.0)
            hnr_sb = rpool.tile([128, NCHP, D], FP8)
            nc.gpsimd.memset(hnr_sb[:, NCH, :], 0.0)

            for _rep in range(reps):
                psAcc_cm = tc.tile_pool(name="psAcc", bufs=1, space="PSUM")
                psAcc = psAcc_cm.__enter__()
                ss_acc = psAcc.tile([NWIN, T], F32, tag="ssacc")
                dd_acc = psAcc.tile([64, T], F32, tag="ddacc")
                with (
                    tc.tile_pool(name="a12", bufs=1) as apool,
                    tc.tile_pool(name="ht", bufs=4) as htpool,
                    tc.tile_pool(name="psA", bufs=2, space="PSUM") as psA,
                    tc.tile_pool(name="psS", bufs=2, space="PSUM") as psS,
                    tc.tile_pool(name="psHV", bufs=2, space="PSUM") as psHV,
                    tc.tile_pool(name="xh", bufs=3) as xhpool,
                ):
                    A1_sb = apool.tile([128, 2, L_LOC], BF16, tag="A1")
                    A2_sb = apool.tile([128, 2, L_LOC], BF16, tag="A2")
                    j0 = 0
                    xh_t = None
                    for blk, jbw in enumerate(BLKS):
                        ht8_t = htpool.tile([128, 8, 512], FP8, tag="ht8")
                        nc.sync.dma_start(
                            ht8_t[:, :, :jbw],
                            ht8[:, j0:j0 + jbw].rearrange("(c p) j -> p c j", p=128))
                        htr_t = htpool.tile([128, 8, 512], FP8, tag="htr")
                        nc.sync.dma_start(
                            htr_t[:, :, :jbw],
                            htr[:, j0:j0 + jbw].rearrange("(c p) j -> p c j", p=128))
                        # A1/A2 for this block: 3-pass fp8 DoubleRow
                        for (w8sb, wrsb, dst, act_copy) in (
                            (wk8_sb, wkr_sb, A1_sb, True),
                            (w28_sb, w2r_sb, A2_sb, False),
                        ):
                            for pc in range(2):
                                ps = psA.tile([128, 512], F32, tag="psA")
                                mm = 0
                                for (wsb, htt) in ((w8sb, ht8_t),
                                                   (wrsb, ht8_t),
                                                   (w8sb, htr_t)):
                                    for s in range(4):
                                        nc.tensor.matmul(
                                            ps[:, :jbw],
                                            wsb[:, 2 * s:2 * s + 2,
                                                pc * 128:(pc + 1) * 128],
                                            htt[:, 2 * s:2 * s + 2, :jbw],
                                            start=(mm == 0), stop=(mm == 11),
                                            perf_mode=DR, skip_group_check=True)
                                        mm += 1
                                if act_copy:
                                    nc.scalar.activation(
                                        dst[:, pc, j0:j0 + jbw], ps[:, :jbw],
                                        AF.Copy, scale=scale8_sb[:])
                                else:
                                    nc.vector.tensor_scalar_mul(
                                        dst[:, pc, j0:j0 + jbw], ps[:, :jbw], 0.25)
                        # PH1 for the chunks of this block
                        c0 = j0 // 128
                        for ci in range(jbw // 128):
                            c = c0 + ci
                            if (c & 1) == 0:
                                xh_t = xhpool.tile([128, 2, T], FP8, tag="xh")
                            ps_s = psS.tile([128, T], F32, tag="psS")
                            for pc in range(2):
                                nc.tensor.matmul(
                                    ps_s[:], A1_sb[:, pc, c * 128:(c + 1) * 128],
                                    qct_sb[:, pc, :],
                                    start=(pc == 0), stop=(pc == 1),
                                    skip_group_check=True)
                            nc.scalar.activation(X_sb[:, c, :], ps_s[:], AF.Exp)
                            ps_hv = psHV.tile([128, T], F32, tag="psHV")
                            for pc in range(2):
                                nc.tensor.matmul(
                                    ps_hv[:], A2_sb[:, pc, c * 128:(c + 1) * 128],
                                    gt_sb[:, pc, :],
                                    start=(pc == 0), stop=(pc == 1),
                                    skip_group_check=True)
                            nc.vector.scalar_tensor_tensor(
                                xh_t[:, c & 1, :], X_sb[:, c, :], 1.0 / 256.0,
                                ps_hv[:], op0=ALU.mult, op1=ALU.mult)
                            nc.tensor.matmul(
                                ss_acc[:], winrow_sb[:, c * NWIN:(c + 1) * NWIN],
                                X_sb[:, c, :],
                                start=(c == 0), stop=(c == NCH - 1),
                                skip_group_check=True)
                            if (c & 1) == 1 or c == NCH - 1:
                                pr = c // 2
                                if c == NCH - 1:
                                    nc.gpsimd.memset(xh_t[:, 1, :], 0.0)
                                nc.tensor.matmul(
                                    dd_acc[:], winrow8_sb[:, pr], xh_t[:],
                                    start=(pr == 0), stop=(pr == NPAIR - 1),
                                    perf_mode=DR, skip_group_check=True)
                        j0 += jbw
                    # natural-layout H for PH3 loads after the ht stream,
                    # so it does not starve the A-phase of DMA bandwidth
                    nc.sync.dma_start(
                        hn8_sb[:, 0:NCH, :],
                        hn8[:].rearrange("(c p) d -> p c d", p=128))
                    nc.sync.dma_start(
                        hnr_sb[:, 0:NCH, :],
                        hnr[:].rearrange("(c p) d -> p c d", p=128))

                # ---- PH2: window scalars + Gamma broadcast
                with tc.tile_pool(name="bcg", bufs=4) as bcgpool:
                    bcg_tiles = []
                    with (
                        tc.tile_pool(name="sc", bufs=1) as scp,
                        tc.tile_pool(name="gamc", bufs=4) as gamcpool,
                        tc.tile_pool(name="psW", bufs=2, space="PSUM") as psW,
                    ):
                        rec_sb = scp.tile([NWIN, T], F32)
                        nc.vector.reciprocal(rec_sb[:], ss_acc[:])
                        lw_sb = scp.tile([NWIN, T], F32)
                        nc.vector.scalar_tensor_tensor(
                            lw_sb[:], dd_acc[:NWIN, :], 8.0, rec_sb[:],
                            op0=ALU.mult, op1=ALU.mult)
                        elw_sb = scp.tile([NWIN, T], F32)
                        nc.scalar.activation(elw_sb[:], lw_sb[:], AF.Exp)
                        gam16 = scp.tile([NWIN, T], BF16)
                        nc.vector.tensor_mul(gam16[:], elw_sb[:], rec_sb[:])
                        gdram = dpool.tile([NCHP, T], BF16)
                        # slab-wise Gamma: small first slab so PH3 starts early
                        q0 = 0
                        for qn in BCG_SLABS:
                            ps_g = psW.tile([12, T], F32, tag="psg")
                            nc.tensor.matmul(
                                ps_g[:qn, :], winT_sb[:, q0:q0 + qn], gam16[:],
                                skip_group_check=True)
                            gamc16 = gamcpool.tile([12, T], BF16,
                                                   tag="gamc")
                            nc.vector.tensor_scalar_mul(
                                gamc16[:qn, :], ps_g[:qn, :], 64.0)
                            nc.sync.dma_start(gdram[q0:q0 + qn, :],
                                              gamc16[:qn, :])
                            bt = bcgpool.tile([128, 12, T], BF16, tag="bcg")
                            nc.gpsimd.dma_start(
                                bt[:, :qn, :],
                                gdram[q0:q0 + qn, :][None, :, :].broadcast_to(
                                    [128, qn, T]))
                            bcg_tiles.append(bt)
                            q0 += qn
                        nc.sync.dma_start(s_out[:], elw_sb[:])
                    psAcc_cm.__exit__(None, None, None)

                    # ---- PH3: z = 64 * (X*Gamma)^T (Hfp8 + Hres), 3-pass DR
                    with (
                        tc.tile_pool(name="zf", bufs=3) as zfpool,
                        tc.tile_pool(name="pp", bufs=5) as pppool,
                        tc.tile_pool(name="pp8", bufs=5) as pp8pool,
                        tc.tile_pool(name="ppr", bufs=5) as pprpool,
                        tc.tile_pool(name="psZ", bufs=1, space="PSUM") as psZ,
                    ):
                        zps = []
                        for tt in range(4):
                            zp = psZ.tile([128, D], F32, tag=f"z{tt}")
                            zps.append(zp)
                        slab_of = []
                        for k, qn in enumerate(BCG_SLABS):
                            slab_of += [k] * qn
                        slab_base = [0, 6, 16, 28]
                        NTAIL = 4        # last pairs run tt-outer so z
                        pps = {}         # streams out while PE still works
                        for pr in range(NPAIR):
                            c2 = 2 * pr
                            kb = slab_of[c2]
                            off = c2 - slab_base[kb]
                            pp_t = pppool.tile([128, 2, T], BF16, tag="pp")
                            nc.vector.tensor_mul(
                                pp_t[:], X_sb[:, c2:c2 + 2, :],
                                bcg_tiles[kb][:, off:off + 2, :])
                            pp8_t = pp8pool.tile([128, 2, T], FP8, tag="pp8")
                            nc.scalar.activation(pp8_t[:], pp_t[:], AF.Copy)
                            ppr_t = pprpool.tile([128, 2, T], FP8, tag="ppr")
                            nc.vector.tensor_sub(ppr_t[:], pp_t[:], pp8_t[:])
                            pps[pr] = (pp8_t, ppr_t)
                            if pr >= NPAIR - NTAIL:
                                continue
                            for tt in range(4):
                                for dn in range(2):
                                    for (st, mv) in (
                                        (pp8_t, hn8_sb), (pp8_t, hnr_sb),
                                        (ppr_t, hn8_sb),
                                    ):
                                        nc.tensor.matmul(
                                            zps[tt][:, dn * 512:(dn + 1) * 512],
                                            st[:, :, tt * 128:(tt + 1) * 128],
                                            mv[:, c2:c2 + 2,
                                               dn * 512:(dn + 1) * 512],
                                            start=(pr == 0 and st is pp8_t
                                                   and mv is hn8_sb),
                                            stop=False,
                                            perf_mode=DR,
                                            skip_group_check=True)
                        for tt in range(4):
                            for pr in range(NPAIR - NTAIL, NPAIR):
                                c2 = 2 * pr
                                pp8_t, ppr_t = pps[pr]
                                lastp = pr == NPAIR - 1
                                for dn in range(2):
                                    for (st, mv) in (
                                        (pp8_t, hn8_sb), (pp8_t, hnr_sb),
                                        (ppr_t, hn8_sb),
                                    ):
                                        nc.tensor.matmul(
                                            zps[tt][:, dn * 512:(dn + 1) * 512],
                                            st[:, :, tt * 128:(tt + 1) * 128],
                                            mv[:, c2:c2 + 2,
                                               dn * 512:(dn + 1) * 512],
                                            start=False,
                                            stop=(lastp and st is ppr_t),
                                            perf_mode=DR,
                                            skip_group_check=True)
                            zf = zfpool.tile([128, D], BF16, tag="zf")
                            if tt % 2 == 0:
                                nc.vector.tensor_copy(zf[:], zps[tt][:])
                            else:
                                nc.scalar.activation(zf[:], zps[tt][:], AF.Copy)
                            zq = (nc.sync, nc.scalar, nc.gpsimd, nc.sync)[tt]
                            zq.dma_start(z_out[tt * 128:(tt + 1) * 128, :],
                                         zf[:])
    nc.compile()
    return nc


_NC_CACHE = None


def _get_nc():
    global _NC_CACHE
    if _NC_CACHE is None:
        _NC_CACHE = _build_bass()
    return _NC_CACHE


def _numpy_fallback(H, G, attn_mask, Wq_core, Wk_core, Wq_win, Wk_win):
    """Reference semantics in numpy; used only if attn_mask has zeros."""
    starts = _window_starts_eff()
    q_t = G @ Wq_win
    scale = D ** -0.5
    out = np.zeros((B, T, D), np.float32)
    for b in range(B):
        m = np.full((T, 1), -np.inf, np.float32)
        ssum = np.zeros((T, 1), np.float32)
        z = np.zeros((T, D), np.float32)
        q = (G[b] @ Wq_core) / np.float32(DP ** 0.5)
        for s0 in starts:
            Hk = H[b, s0:s0 + WIN, :]
            mk = attn_mask[b, s0:s0 + WIN]
            k = Hk @ Wk_core
            sc = q @ k.T
            sc = np.where(mk[None, :], sc, np.float32(-1e30))
            sc -= sc.max(axis=-1, keepdims=True)
            al = np.exp(sc)
            al /= al.sum(axis=-1, keepdims=True)
            Zk = al @ Hk
            k_w = Zk @ Wk_win
            lw = (q_t[b] * k_w).sum(-1, keepdims=True) * scale
            m_new = np.maximum(m, lw)
            em, ew = np.exp(m - m_new), np.exp(lw - m_new)
            ssum = ssum * em + ew
            z = z * em + ew * Zk
            m = m_new
        out[b] = z / (ssum + 1e-8)
    return out


def _fp8_split(x):
    """x (f32) -> (fp8 main, fp8 residual); main+res reconstructs x closely."""
    m = x.astype(E4)
    r = (x - m.astype(np.float32)).astype(E4)
    return m, r


def kernel(H, G, attn_mask, Wq_core, Wk_core, Wq_win, Wk_win):
    H = np.asarray(H, np.float32)
    G = np.asarray(G, np.float32)
    Wq_core = np.asarray(Wq_core, np.float32)
    Wk_core = np.asarray(Wk_core, np.float32)
    Wq_win = np.asarray(Wq_win, np.float32)
    Wk_win = np.asarray(Wk_win, np.float32)
    mask = np.asarray(attn_mask)
    if not mask.all():
        return _numpy_fallback(H, G, mask, Wq_core, Wk_core, Wq_win, Wk_win)

    halves = _core_plan()
    bf = ml_dtypes.bfloat16
    wk8_h, wkr_h = _fp8_split(8.0 * Wk_core)
    w2 = Wk_win @ Wq_win.T                                  # [D, DG]
    w28_h, w2r_h = _fp8_split(4.0 * w2)

    in_maps = []
    for b in range(B):
        q_coreT = np.ascontiguousarray((G[b] @ Wq_core).T / 16.0).astype(bf)
        GT_b = np.ascontiguousarray(G[b].T).astype(bf)
        for h in halves:
            wloc = h["win_local"]
            nwin = len(wloc)
            win = np.zeros((NCHP, NWIN), np.float32)
            for w, cw in enumerate(wloc):
                win[cw:cw + 12, w] = 1.0
            winT = np.ascontiguousarray(win.T)   # dummy cols all zero
            # dummy window columns get a harmless nonzero row so the window
            # sum E stays finite; winT zeros keep them out of Gamma, and the
            # host ignores their s_out rows.
            win[NCH - 1, nwin:] = 1.0
            winrow = np.zeros((128, NCH * NWIN), np.float32)
            for c in range(NCH):
                winrow[:, c * NWIN:(c + 1) * NWIN] = win[c]
            winrow8 = np.zeros((128, NPAIR * 2 * 64), np.float32)
            for pr in range(NPAIR):
                for i in range(2):
                    winrow8[:, pr * 128 + i * 64:pr * 128 + i * 64 + NWIN] = \
                        win[2 * pr + i]
            Hs = np.ascontiguousarray(H[b, h["lo"]:h["lo"] + L_LOC, :])
            hn8_h, hnr_h = _fp8_split(Hs)
            in_maps.append(dict(
                ht8=np.ascontiguousarray(hn8_h.T),
                htr=np.ascontiguousarray(hnr_h.T),
                hn8=hn8_h, hnr=hnr_h,
                qct=q_coreT, gt=GT_b,
                wk8=wk8_h, wkr=wkr_h, w28=w28_h, w2r=w2r_h,
                winT=winT.astype(bf),
                winrow=winrow.astype(bf), winrow8=winrow8.astype(E4)))

    global _last_in_maps
    _last_in_maps = in_maps
    nc = _get_nc()
    res = run_bass_kernel_spmd(nc, in_maps, core_ids=list(range(8)))
    out = np.zeros((B, T, D), np.float32)
    nw0 = len(halves[0]["win_local"])
    nw1 = len(halves[1]["win_local"])
    for b in range(B):
        r0, r1 = res.results[2 * b], res.results[2 * b + 1]
        denom = (r0["s_out"][:nw0].sum(axis=0) + r1["s_out"][:nw1].sum(axis=0)
                 + 1e-8)
        out[b] = (r0["z_out"].astype(np.float32)
                  + r1["z_out"].astype(np.float32)) / 64.0 / denom[:, None]
    return out
